# revision 25
# baseline (speedup 1.0000x reference)
"""Trainium2 Bass/Tile kernel: fused fp8-quantized multi-head causal attention.

Module: q/k/v = fp8(x) @ fp8(W) + b ; scores = (q k^T)/sqrt(64) with causal
mask (-1000 => exp underflows to exactly 0) ; out = softmax(scores) @ v @ W_O + b_O.

Sharding (8 NeuronCores, SPMD, no collectives):
  core c -> batch b = c // 4, head group hg = c % 4 (heads 4*hg .. 4*hg+3).
  Each core returns a partial [S, M] bf16 output (its 4 heads' contribution);
  the host sums the 4 partials per batch (fp32) and adds b_O.

Host-side preprocessing: inputs/W_{Q,K,V} are quantized to fp8-e4m3 on the
host and uploaded in the exact SBUF layout so every DMA is fully contiguous.

Key design points of this version:
  * softmax exp is SPLIT across three engines per (sk-chunk, head):
    ScalarE runs the exact table exp; VectorE and GpSimd run a
    Schraudolph-style exp (uint16(x*128/ln2 + 16256) bit-cast to bf16,
    ~±3% relative, validated to keep end-to-end rel err < 1e-2) via a
    single tensor_scalar each.  This removes the ScalarE ACTIVATE
    bottleneck (~82 us in the previous version).
  * scores psums are single-bank per (chunk, head) tiles (pool bufs=4),
    filler matmuls get their own 1-bank pool, z psums 3 bufs -> 8 banks.
  * qT/kT/outproj psum evictions moved to ScalarE activation
    (Identity with per-partition bias AP / Copy), freeing the DVE.
  * weight tensors are separate DRAM params/tiles so the first projection
    only waits on its own weight DMA; DMA order puts the q/k-path first.
"""

import os
import sys

for _p in ("/opt/trn_rl_repo", os.path.expanduser("~/.axon_site/_ro/trn_rl_repo")):
    if os.path.isdir(_p) and _p not in sys.path:
        sys.path.insert(0, _p)

import ml_dtypes
import numpy as np

import concourse.bass as bass
import concourse.mybir as mybir
import concourse.tile as tile
from concourse import bacc
from concourse.bass_utils import run_bass_kernel_spmd

B, S, M, H, D = 2, 2048, 1024, 16, 64
HG = 4                 # heads per core
NCORES = 8
SQ = 512               # sq chunk width (one fp32 psum bank)
NSQ = S // SQ          # 4
NMC = M // 128         # 8 contraction chunks for projections
NSS = S // 128         # 16 s sub-chunks of 128

F8 = mybir.dt.float8e4
BF = mybir.dt.bfloat16
F32 = mybir.dt.float32
U16 = mybir.dt.uint16
EXP = mybir.ActivationFunctionType.Exp
IDENT = mybir.ActivationFunctionType.Identity
COPYF = mybir.ActivationFunctionType.Copy
DR = mybir.MatmulPerfMode.DoubleRow

_f8 = ml_dtypes.float8_e4m3
_bf16 = ml_dtypes.bfloat16

# Schraudolph exp -> bf16 bit pattern: uint16(x * 128/ln2 + (16256 + delta))
EXPA = 184.6650390625
EXPB = 16256.0 - 6.0

# exp engine schedule: cycle over these per (chunk, head).
# NOTE: gpsimd cannot read PSUM, so only v (DVE Schraudolph) and
# a (ScalarE exact table exp) are usable here.
EXP_ENGINES = os.environ.get("KEXP", "av")


def _build_nc():
    nc = bacc.Bacc(
        "TRN2", target_bir_lowering=False, debug=False, num_devices=NCORES
    )

    xq = nc.declare_dram_parameter("xq_t8", [128, NSQ, NMC, SQ], F8, isOutput=False)
    xk = nc.declare_dram_parameter("xk_t8", [128, NSQ, NMC, SQ], F8, isOutput=False)
    xv = nc.declare_dram_parameter("xv_t8", [128, NSQ, NMC, SQ], F8, isOutput=False)
    wqd = nc.declare_dram_parameter("wq8", [128, NMC * HG * D], F8, isOutput=False)
    wkd = nc.declare_dram_parameter("wk8", [128, NMC * HG * D], F8, isOutput=False)
    wvd = nc.declare_dram_parameter("wv8", [128, NMC * HG * D], F8, isOutput=False)
    wo = nc.declare_dram_parameter("wo_bf", [128, 2 * M], BF, isOutput=False)
    bqk = nc.declare_dram_parameter("bqk", [128, 4], F32, isOutput=False)
    bv = nc.declare_dram_parameter("bv", [1, HG * D], F32, isOutput=False)
    out_p = nc.declare_dram_parameter("out_p", [S, M], BF, isOutput=True)

    with tile.TileContext(nc) as tc:
        with (
            tc.tile_pool(name="persist", bufs=1) as pers,
            tc.tile_pool(name="work", bufs=2) as work,
            tc.tile_pool(name="pps", bufs=2, space="PSUM") as pps,
            tc.tile_pool(name="ppf", bufs=2, space="PSUM") as ppf,
            tc.tile_pool(name="ppz", bufs=2, space="PSUM") as ppz,
        ):
            # ---- persistent SBUF tensors ----
            xq_sb = pers.tile([128, NSQ, NMC, SQ], F8, tag="xq")
            xk_sb = pers.tile([128, NSQ, NMC, SQ], F8, tag="xk")
            xv_sb = pers.tile([128, NSQ, NMC, SQ], F8, tag="xv")
            wq_sb = pers.tile([128, NMC, HG * D], F8, tag="wq")
            wk_sb = pers.tile([128, NMC, HG * D], F8, tag="wk")
            wv_sb = pers.tile([128, NMC, HG * D], F8, tag="wv")
            wo_sb = pers.tile([128, 2, M], BF, tag="wo")
            bqk_sb = pers.tile([128, 4], F32, tag="bqk")
            bv_sb = pers.tile([1, HG * D], F32, tag="bv")
            bvb_sb = pers.tile([128, HG, D], F32, tag="bvb")
            qt_sb = pers.tile([128, 2, S], BF, tag="qt")
            kt_sb = pers.tile([128, 2, S], BF, tag="kt")
            zt_sb = pers.tile([128, 2, S], BF, tag="zt")
            v_sb = pers.tile([128, NSS, HG, D + 1], BF, tag="v")
            expwarm = pers.tile([1, 1], F32, tag="expwarm")

            # ---- input DMAs: q/k path first so the first projections can
            # start as soon as their own operands land ----
            wq4 = wqd[:, :].rearrange("p (c d) -> p c d", c=NMC)
            wk4 = wkd[:, :].rearrange("p (c d) -> p c d", c=NMC)
            wv4 = wvd[:, :].rearrange("p (c d) -> p c d", c=NMC)
            nc.scalar.dma_start(out=bqk_sb[:, :], in_=bqk[:, :])
            nc.scalar.dma_start(out=bv_sb[:, :], in_=bv[:, :])
            nc.scalar.dma_start(out=wq_sb[:, :, :], in_=wq4)
            nc.sync.dma_start(out=xq_sb[:, 0], in_=xq[:, 0])
            nc.scalar.dma_start(out=wk_sb[:, :, :], in_=wk4)
            nc.sync.dma_start(out=xk_sb[:, 0], in_=xk[:, 0])
            nc.sync.dma_start(out=xv_sb[:, 0], in_=xv[:, 0])
            nc.scalar.dma_start(out=wv_sb[:, :, :], in_=wv4)
            for t in range(1, NSQ):
                for x_sb, x_dram in ((xq_sb, xq), (xk_sb, xk), (xv_sb, xv)):
                    nc.sync.dma_start(out=x_sb[:, t], in_=x_dram[:, t])
            nc.scalar.dma_start(
                out=wo_sb[:, :, :], in_=wo[:, :].rearrange("p (c m) -> p c m", c=2)
            )

            # warm the exp table set while the DMAs run
            nc.gpsimd.memset(expwarm[:, :], 1.0)
            nc.gpsimd.memset(v_sb[:, :, :, D : D + 1], 1.0)
            nc.scalar.activation(expwarm[:, :], expwarm[:, :], EXP)

            # keep the PE HAM activity monitor busy while the first DMAs
            # land so the real matmuls start at the warm (2.4 GHz) clock
            warm_bf = pers.tile([1, 256], BF, tag="warm_bf")
            nc.gpsimd.memset(warm_bf[:, :], 1.0)
            ones_f = pers.tile([1, D], F32, tag="ones_f")
            nc.gpsimd.memset(ones_f[:, :], 1.0)

            # broadcast b_V across partitions for the v-eviction add
            nc.gpsimd.partition_broadcast(
                bvb_sb.rearrange("p g d -> p (g d)"), bv_sb[0:1, :]
            )

            ps_w2 = ppf.tile([128, SQ], F32, tag="pf", name="ps_warm")
            for _ in range(34):
                nc.tensor.matmul(
                    ps_w2[0:64, 0:256],
                    lhsT=warm_bf[0:1, 0:64],
                    rhs=warm_bf[0:1, :],
                    start=True,
                    stop=True,
                )

            # ---- exp emission: split across engines ----
            exp_counter = [0]

            def emit_exp(dst_bf_ap, ps_ap):
                e = EXP_ENGINES[exp_counter[0] % len(EXP_ENGINES)]
                exp_counter[0] += 1
                if e == "a":
                    nc.scalar.activation(dst_bf_ap, ps_ap, EXP)
                else:
                    eng = nc.vector if e == "v" else nc.gpsimd
                    eng.tensor_scalar(
                        out=dst_bf_ap.bitcast(U16),
                        in0=ps_ap,
                        scalar1=EXPA,
                        scalar2=EXPB,
                        op0=mybir.AluOpType.mult,
                        op1=mybir.AluOpType.add,
                    )

            # ---- filler units (emitted inside the attention loop) ----
            def projqk_unit(dst_sb, w_sb, x_sb, bcol, t, c, scale=1.0):
                """qT/kT projection for s-window t, head-pair half c; the
                bias add (and the 1/8 attention scale for k) happens in a
                ScalarE Identity-activation eviction."""

                def emit():
                    ssl = slice(SQ * t, SQ * t + SQ)
                    ps = ppf.tile([128, SQ], F32, tag="pf", name=f"pj{t}_{c}_{bcol}")
                    for mi in range(0, NMC, 2):
                        nc.tensor.matmul(
                            ps[:, :],
                            lhsT=w_sb[:, mi : mi + 2, 128 * c : 128 * c + 128],
                            rhs=x_sb[:, t, mi : mi + 2, :],
                            start=(mi == 0),
                            stop=(mi == NMC - 2),
                            perf_mode=DR,
                        )
                    nc.scalar.activation(
                        dst_sb[:, c, ssl],
                        ps[:, :],
                        IDENT,
                        bias=bqk_sb[:, bcol : bcol + 1],
                        scale=scale,
                    )

                return emit

            def projv_unit(ss):
                """v projection for s-subchunk ss (128 rows)."""

                def emit():
                    ps = ppf.tile([128, SQ], F32, tag="pf", name=f"pv{ss}")
                    for mi in range(0, NMC, 2):
                        nc.tensor.matmul(
                            ps[:, 0 : HG * D],
                            lhsT=xv_sb[:, ss // 4, mi : mi + 2, 128 * (ss % 4) : 128 * (ss % 4) + 128],
                            rhs=wv_sb[:, mi : mi + 2, :],
                            start=(mi == 0),
                            stop=(mi == NMC - 2),
                            perf_mode=DR,
                        )
                    nc.vector.tensor_tensor(
                        out=v_sb[:, ss, :, 0:D],
                        in0=ps[:, 0 : HG * D].rearrange("p (g d) -> p g d", g=HG),
                        in1=bvb_sb[:, :, :],
                        op=mybir.AluOpType.add,
                    )

                return emit

            def outproj_unit(jq, ss4, n, evict=None):
                """output projection for rows [512*jq + 128*ss4, +128),
                columns [512*n, +512)."""

                def emit():
                    psl = slice(SQ * jq + 128 * ss4, SQ * jq + 128 * ss4 + 128)
                    nsl = slice(SQ * n, SQ * n + SQ)
                    o16 = work.tile([128, SQ], BF, tag="o16", bufs=3,
                                    name=f"o{jq}_{ss4}_{n}")
                    ps_o = ppf.tile([128, SQ], F32, tag="pf",
                                    name=f"po{jq}_{ss4}_{n}")
                    for c in range(2):
                        nc.tensor.matmul(
                            ps_o[:, :],
                            lhsT=zt_sb[:, c, psl],
                            rhs=wo_sb[:, c, nsl],
                            start=(c == 0),
                            stop=(c == 1),
                        )
                    ev = evict
                    if ev is None and ss4 % 2 == 1 and n == 1:
                        ev = "vector"
                    if ev == "vector":
                        nc.vector.tensor_copy(o16[:, :], ps_o[:, :])
                    else:
                        nc.scalar.activation(o16[:, :], ps_o[:, :], COPYF)
                    dma_eng = nc.sync if n == 0 else nc.scalar
                    dma_eng.dma_start(out=out_p[psl, nsl], in_=o16[:, :])

                return emit

            def emit_z(ps_z, c, prev, jq, last):
                """z += v.T @ pattern for one sk-chunk. On diagonal-band
                chunks the causally-full columns go first (they do not wait
                for the gpsimd mask); the masked diagonal block follows."""
                pp, psi, pw0 = prev
                band = psi >= 4 * jq
                ranges = []
                if band and pw0 + 128 < SQ:
                    ranges.append((pw0 + 128, SQ, False))
                if band:
                    ranges.append((pw0, pw0 + 128, True))
                if not band:
                    ranges.append((pw0, SQ, False))
                for u in range(2):
                    for ri, (lo, hi, _) in enumerate(ranges):
                        nc.tensor.matmul(
                            ps_z[u][:, lo:hi],
                            lhsT=v_sb[:, psi, 2 * c + u, :],
                            rhs=pp[:, u, lo:hi],
                            start=(psi == 0 and ri == 0),
                            stop=(last and ri == len(ranges) - 1),
                        )

            # ---- fused schedule ----
            # minimal pre-loop: just q/k window 0 half 0 before jq=0
            projqk_unit(qt_sb, wq_sb, xq_sb, 0, 0, 0)()
            projqk_unit(kt_sb, wk_sb, xk_sb, 2, 0, 0, 0.125)()

            for jq in range(NSQ):
                qsl = slice(SQ * jq, SQ * jq + SQ)
                nsk = 4 * (jq + 1)
                # deadline-driven filler assignment:
                #   c0 slot i (i<4): v chunk 4*jq+i (first used by z at
                #     absolute chunk 4*jq+i, i.e. c0 slot 4*jq+i+1)
                #   c0 later slots + c1 slots: next window's q/k and the
                #     previous window's output projection
                fill = {0: {}, 1: {}}
                for i in range(4):
                    fill[0].setdefault(i, []).append(projv_unit(4 * jq + i))
                if jq == 0:
                    fill[0].setdefault(1, []).append(
                        projqk_unit(qt_sb, wq_sb, xq_sb, 1, 0, 1))
                    fill[0].setdefault(2, []).append(
                        projqk_unit(kt_sb, wk_sb, xk_sb, 3, 0, 1, 0.125))
                rest0, rest1 = [], []
                if jq > 0:
                    oph = [outproj_unit(jq - 1, ss4, n)
                           for ss4 in range(4) for n in range(2)]
                    rest0 += oph[:4]
                    rest1 += oph[4:]
                if jq < NSQ - 1:
                    t = jq + 1
                    rest1 = [
                        projqk_unit(qt_sb, wq_sb, xq_sb, 0, t, 0),
                        projqk_unit(kt_sb, wk_sb, xk_sb, 2, t, 0, 0.125),
                        projqk_unit(qt_sb, wq_sb, xq_sb, 1, t, 1),
                        projqk_unit(kt_sb, wk_sb, xk_sb, 3, t, 1, 0.125),
                    ] + rest1
                for lst, cc in ((rest0, 0), (rest1, 1)):
                    free = nsk - 4 if cc == 0 else nsk
                    base = 4 if cc == 0 else 0
                    for i, u in enumerate(lst):
                        s = base + (i * free // len(lst) if free > 0 else i)
                        fill[cc].setdefault(min(s, max(nsk - 2, 0)), []).append(u)

                for c in range(2):  # head pair: heads (2c, 2c+1)
                    ps_z = [
                        ppz.tile([D + 1, SQ], F32, tag="ppz", name=f"psz{jq}_{c}_{u}")
                        for u in range(2)
                    ]
                    prev = None  # delayed-z pipeline: (p_bf, si, w0)
                    for si in range(nsk):
                        ksl = slice(128 * si, 128 * si + 128)
                        r = si - 4 * jq  # >=0 on diagonal-band tiles
                        w0 = 128 * r if r > 0 else 0  # fully-masked prefix
                        # both heads' scores into one 2-bank psum tile
                        ps2 = pps.tile([128, 2, SQ], F32, tag="ps2",
                                       name=f"s{jq}_{c}_{si}")
                        for u in range(2):
                            hsl = slice(64 * u, 64 * u + 64)
                            nc.tensor.matmul(
                                ps2[:, u, w0:SQ],
                                lhsT=kt_sb[hsl, c, ksl],
                                rhs=qt_sb[hsl, c, SQ * jq + w0 : SQ * jq + SQ],
                                start=True,
                                stop=True,
                            )
                        p_bf = work.tile([128, 2, SQ], BF, tag="p", bufs=4,
                                         name=f"p{jq}_{c}_{si}")
                        emit_exp(p_bf[:, :, w0:SQ], ps2[:, :, w0:SQ])
                        if r >= 0:
                            # in-place triangular mask on the diagonal block,
                            # both heads in one gpsimd op: keep col >= row
                            nc.gpsimd.affine_select(
                                out=p_bf[:, :, w0 : w0 + 128],
                                in_=p_bf[:, :, w0 : w0 + 128],
                                compare_op=mybir.AluOpType.is_ge,
                                fill=0.0,
                                base=0,
                                pattern=[[0, 2], [1, 128]],
                                channel_multiplier=-1,
                            )
                        if prev is not None:
                            emit_z(ps_z, c, prev, jq, last=False)
                        prev = (p_bf, si, w0)
                        for emit in fill[c].get(si, ()):
                            emit()
                    # drain the pipelined z for the last sk-chunk
                    emit_z(ps_z, c, prev, jq, last=True)
                    # normalize both heads of the pair (interleaved chains)
                    tail = jq == NSQ - 1 and c == 1
                    recips, rbs = [], []
                    for u in range(2):
                        dn = work.tile([1, SQ], F32, tag="dn", name=f"dn{jq}{c}{u}")
                        nc.scalar.activation(dn[:, :], ps_z[u][D : D + 1, :], COPYF)
                        recip = work.tile([1, SQ], F32, tag="recip",
                                          name=f"rc{jq}{c}{u}")
                        nc.vector.reciprocal_approx_fast(
                            out=recip[:, :], in_=dn[:, :]
                        )
                        recips.append(recip)
                        rb = work.tile([D, SQ], F32, tag="rb",
                                       name=f"rb{jq}{c}{u}")
                        nc.gpsimd.partition_broadcast(rb[:, :], recip[0:1, :])
                        rbs.append(rb)
                    if tail:
                        # fine-grained tail: normalize per 128-row slice and
                        # start that slice's output projection immediately
                        for ss4 in range(4):
                            fsl = slice(128 * ss4, 128 * ss4 + 128)
                            for u in range(2):
                                nc.vector.tensor_mul(
                                    zt_sb[64 * u : 64 * u + 64, c,
                                          SQ * jq + 128 * ss4 : SQ * jq + 128 * ss4 + 128],
                                    ps_z[u][0:D, fsl],
                                    rbs[u][:, fsl],
                                )
                            outproj_unit(jq, ss4, 0)()
                            outproj_unit(jq, ss4, 1)()
                    else:
                        for u in range(2):
                            nc.vector.tensor_mul(
                                zt_sb[64 * u : 64 * u + 64, c, qsl],
                                ps_z[u][0:D, :],
                                rbs[u][:, :],
                            )

    if not nc.is_finalized():
        nc.finalize()
    return nc


_NC = None


def _get_nc():
    global _NC
    if _NC is None:
        _NC = _build_nc()
    return _NC


def _wpack(w):
    """[M, HG*D] -> partition-major [128, NMC*HG*D] (2 KiB contiguous rows)."""
    return np.ascontiguousarray(
        w.reshape(NMC, 128, HG * D).transpose(1, 0, 2).reshape(128, NMC * HG * D)
    )


def _make_in_maps(inputs):
    q8 = lambda a: np.asarray(a, np.float32).astype(_f8)
    xt = {}
    for name, key in (("xq_t8", "query_input"), ("xk_t8", "key_input"),
                      ("xv_t8", "value_input")):
        # [S, M] -> fp8 [M, S] -> [p=128, t=4, mi=8, s'=512] (SBUF layout)
        xt[name] = [
            np.ascontiguousarray(
                q8(inputs[key][b]).T.reshape(NMC, 128, NSQ, SQ).transpose(1, 2, 0, 3)
            )
            for b in range(B)
        ]

    wq8 = q8(inputs["W_Q"])  # [H, M, D]
    wk8 = q8(inputs["W_K"])
    wv8 = q8(inputs["W_V"])
    wo = np.asarray(inputs["W_O"], np.float32)  # [H, D, M]

    in_maps = []
    for core in range(NCORES):
        b, hg = core // HG, core % HG
        hs = slice(HG * hg, HG * hg + HG)
        # k bias pre-scaled by 0.125: the kT eviction computes
        # (psum * 0.125 + bias), so bias must carry the scale too.
        bq_col = np.asarray(inputs["b_Q"], np.float32)[hs].reshape(2, 128).T
        bk_col = np.asarray(inputs["b_K"], np.float32)[hs].reshape(2, 128).T * 0.125
        m = {
            "xq_t8": xt["xq_t8"][b],
            "xk_t8": xt["xk_t8"][b],
            "xv_t8": xt["xv_t8"][b],
            "wq8": _wpack(wq8[hs].transpose(1, 0, 2).reshape(M, HG * D)),
            "wk8": _wpack(wk8[hs].transpose(1, 0, 2).reshape(M, HG * D)),
            "wv8": _wpack(wv8[hs].transpose(1, 0, 2).reshape(M, HG * D)),
            "wo_bf": np.ascontiguousarray(
                wo[hs]
                .reshape(HG * D, M)
                .astype(_bf16)
                .reshape(2, 128, M)
                .transpose(1, 0, 2)
                .reshape(128, 2 * M)
            ),
            "bqk": np.ascontiguousarray(np.concatenate([bq_col, bk_col], axis=1)),
            "bv": np.asarray(inputs["b_V"], np.float32)[hs].reshape(1, HG * D).copy(),
        }
        in_maps.append(m)
    return in_maps


def _run(inputs, **kw):
    nc = _get_nc()
    in_maps = _make_in_maps(inputs)
    res = run_bass_kernel_spmd(nc, in_maps, list(range(NCORES)), **kw)
    out = np.zeros((B, S, M), np.float32)
    for core in range(NCORES):
        out[core // HG] += res.results[core]["out_p"].astype(np.float32)
    out += np.asarray(inputs["b_O"], np.float32)
    return out, res


def kernel(**inputs):
    out, _ = _run(inputs)
    return out
